# revision 25
# baseline (speedup 1.0000x reference)
"""GroupQueryAttention TRN2 Bass kernel (v4: double-block exp pipeline).

Problem: B=4, T=2048, C=1024, H=16 heads, G=4 groups, head_dim=64, causal.
Sharding: 8 cores = 4 batches (DP) x 2 tensor-parallel halves (8 heads /
2 groups each). Host pre-transposes x and weight slices; each core computes
a partial output projection over its 512 attention channels; host sums the
two TP partials per batch and adds the bias.

Hardware model (microbenchmarked on this part):
  - back-to-back matmuls sustain ~2.3 rows/ns (2.4GHz) regardless of
    dtype and of concurrent ACT/DVE/DMA load; but any PE stall drops the
    clock and it takes ~3us of continuous work to ramp back;
  - a sustained [128,512] psum->sbuf exp costs ~690ns (not 530): the ACT
    instruction overhead is ~200ns, so exp work across the causal area
    (~320 blocks) is ~220us -- ABOVE the PE's 186us -- unless batched.

Design:
  - score matmuls write TWO tk blocks into one [128,1024] psum tile
    (2 banks; each matmul stays within a bank) and ONE exp covers both
    blocks -> ACT drops to ~160us. The clipped diagonal leaves a garbage
    gap inside the exp range; its exp output is never read by the PVs.
  - TWO heads processed block-interleaved; PVs consume the exp of the
    PREVIOUS double-block, so the PE never waits on ACT.
  - psum: psS 2x[128,1024] (4 banks) + psO 2 + psProj 1 + psY 1.
  - projections(j+1) are chopped into single-matmul filler slots pulled
    between double-groups; ALL out-projection is deferred and injected
    during attention(3) (which otherwise is ACT-bound), the rest after.
  - ACT runs ONLY exps. DVE does all psum->sbuf copies, diag masks,
    normalize, y copies.
  - dtypes: projections fp32r, attention bf16 (rel err ~1.6e-3).
  - PV lhsT v tiles are [ones64 | v64] so PV emits the softmax
    denominator rows at zero extra PE cost.
"""

import sys
import numpy as np
import ml_dtypes

for _p in ("/opt/trn_rl_repo", "/opt/trn_rl_repo/concourse"):
    if _p not in sys.path:
        sys.path.insert(0, _p)

import concourse.bass as bass  # noqa: E402
import concourse.mybir as mybir  # noqa: E402
from concourse import bacc  # noqa: E402
from concourse.tile import TileContext  # noqa: E402
from concourse.bass_utils import run_bass_kernel_spmd  # noqa: E402
from concourse.masks import make_identity, make_upper_triangular  # noqa: E402

F32 = mybir.dt.float32
F32R = mybir.dt.float32r
BF16 = mybir.dt.bfloat16
BF = ml_dtypes.bfloat16

B, T, C = 4, 2048, 1024
NH, NG, HD = 16, 4, 64
NH_LOC, NG_LOC = 8, 2
S = NH_LOC * HD
TQB = 512
NTQB = 4
NKT = 16
NCT = 8
SCALE = float(HD) ** -0.5
EXP = mybir.ActivationFunctionType.Exp


def _build_program():
    nc = bacc.Bacc("TRN2", target_bir_lowering=False, debug=False, num_devices=8)

    xT = nc.dram_tensor("xT", [C, T], F32R, kind="ExternalInput")
    wqT = nc.dram_tensor("wqT", [C, S], F32R, kind="ExternalInput")
    wkT = nc.dram_tensor("wkT", [C, NG_LOC * HD], F32R, kind="ExternalInput")
    wvT = nc.dram_tensor("wvT", [C, NG_LOC * HD], F32R, kind="ExternalInput")
    wpT = nc.dram_tensor("wpT", [S, C], F32R, kind="ExternalInput")
    y = nc.dram_tensor("y", [T, C], F32, kind="ExternalOutput")

    with TileContext(nc) as tc:
        with tc.tile_pool(name="const", bufs=1) as const_pool, \
             tc.tile_pool(name="persist", bufs=1) as persist, \
             tc.tile_pool(name="vtp", bufs=2) as vtp, \
             tc.tile_pool(name="pp", bufs=5) as ppool, \
             tc.tile_pool(name="attn", bufs=2) as apool, \
             tc.tile_pool(name="sm", bufs=4) as small, \
             tc.tile_pool(name="yo", bufs=4) as ypool, \
             tc.tile_pool(name="psProj", bufs=1, space="PSUM") as psProj, \
             tc.tile_pool(name="psS", bufs=2, space="PSUM") as psS, \
             tc.tile_pool(name="psO", bufs=2, space="PSUM") as psO, \
             tc.tile_pool(name="psY", bufs=1, space="PSUM") as psY:

            # ---- constants ----
            ident = const_pool.tile([128, 64], F32)
            make_identity(nc, ident[0:64, 0:64])
            make_identity(nc, ident[64:128, 0:64], nomemset=False)
            # additive causal bias for diagonal blocks: 0 where tq>=tk,
            # -240 otherwise (exp(-240*0.125)=e^-30 ~ 0); applied to the
            # psum scores BEFORE exp so the PVs depend only on the exp
            mask32 = const_pool.tile([128, 128], F32)
            make_upper_triangular(nc, mask32, val=1.0, diag=True)
            mbias = const_pool.tile([128, 128], F32)
            nc.vector.tensor_scalar_mul(mbias, mask32, 240.0)
            nc.vector.tensor_scalar_add(mbias, mbias, -240.0)

            # ---- persistent SBUF ----
            xAt = [persist.tile([128, (NCT // 2) * TQB], F32R, tag=f"xa{hf}",
                                name=f"xa{hf}") for hf in range(2)]
            xBt = [persist.tile([128, (NCT // 2) * (T - TQB)], F32R, tag=f"xb{hf}",
                                name=f"xb{hf}") for hf in range(2)]
            wq_t = persist.tile([128, NCT * S], F32R, tag="wq", name="wq_t")
            wk_t = persist.tile([128, NCT * NG_LOC * HD], F32R, tag="wk", name="wk_t")
            wv_t = persist.tile([128, NCT * NG_LOC * HD], F32R, tag="wv", name="wv_t")
            wp_t = persist.tile([128, 4 * C], F32R, tag="wp", name="wp_t")
            wq_sb = [wq_t[:, ct * S:(ct + 1) * S] for ct in range(NCT)]
            wk_sb = [wk_t[:, ct * 128:(ct + 1) * 128] for ct in range(NCT)]
            wv_sb = [wv_t[:, ct * 128:(ct + 1) * 128] for ct in range(NCT)]
            wp_sb = [wp_t[:, i * C:(i + 1) * C] for i in range(4)]
            qt_sb = [persist.tile([128, T], BF16, tag=f"qt{i}", name=f"qt{i}")
                     for i in range(4)]
            kdup = [persist.tile([128, T], BF16, tag=f"kd{g}", name=f"kd{g}")
                    for g in range(NG_LOC)]
            v_sb = [persist.tile([128, NKT * 128], BF16, tag=f"v{g}", name=f"v{g}")
                    for g in range(NG_LOC)]
            for g in range(NG_LOC):
                nc.vector.memset(v_sb[g], 1.0)

            # ---- input DMAs (consolidated: Sync issues ~600ns each) ----
            # dram [nct*128, W] -> sbuf [128, (nct W)] with c = ct*128 + p
            def _ld(dst, drsrc, nct):
                nc.sync.dma_start(
                    out=dst[:, :].rearrange("p (ct w) -> p ct w", ct=nct),
                    in_=drsrc.rearrange("(ct p) w -> p ct w", ct=nct))

            _ld(wk_t, wkT[:, :], NCT)
            _ld(wv_t, wvT[:, :], NCT)
            _ld(xAt[0], xT[0:512, 0:TQB], 4)
            _ld(xAt[1], xT[512:1024, 0:TQB], 4)
            _ld(wq_t, wqT[:, :], NCT)
            _ld(xBt[0], xT[0:512, TQB:T], 4)
            _ld(xBt[1], xT[512:1024, TQB:T], 4)
            _ld(wp_t, wpT[:, :], 4)

            def xcol(ct, j):
                if j == 0:
                    return xAt[ct // 4][:, (ct % 4) * TQB:(ct % 4 + 1) * TQB]
                w = T - TQB
                base = (ct % 4) * w + (j - 1) * TQB
                return xBt[ct // 4][:, base:base + TQB]

            at_tiles = {}

            # ---------------- filler slot builders ----------------
            def proj_slots(j):
                slots = []
                cols = slice(j * TQB, (j + 1) * TQB)
                box = {}

                def k_mm(ct):
                    def f():
                        if ct == 0:
                            box['k'] = psProj.tile([128, TQB], F32, tag="pj",
                                                   name=f"psk{j}")
                        psk = box['k']
                        nc.tensor.matmul(psk, wk_sb[ct], xcol(ct, j),
                                         start=(ct == 0), stop=(ct == NCT - 1))
                        if ct == NCT - 1:
                            nc.vector.tensor_copy(kdup[0][0:64, cols], psk[0:64, :])
                            nc.vector.tensor_copy(kdup[1][64:128, cols],
                                                  psk[64:128, :])
                            nc.sync.dma_start(out=kdup[0][64:128, cols],
                                              in_=kdup[0][0:64, cols])
                            nc.sync.dma_start(out=kdup[1][0:64, cols],
                                              in_=kdup[1][64:128, cols])
                    return f

                def v_mm(ct):
                    def f():
                        if ct == 0:
                            box['v'] = psProj.tile([128, TQB], F32, tag="pj",
                                                   name=f"psv{j}")
                        psv = box['v']
                        nc.tensor.matmul(psv, wv_sb[ct], xcol(ct, j),
                                         start=(ct == 0), stop=(ct == NCT - 1))
                        if ct == NCT - 1:
                            box['vt'] = vtp.tile([128, TQB], F32, tag="vt",
                                                 name=f"vt{j}")
                            nc.vector.tensor_copy(box['vt'], psv)
                    return f

                def q_mm(p4, ct):
                    def f():
                        if ct == 0:
                            box[f'q{p4}'] = psProj.tile([128, TQB], F32, tag="pj",
                                                        name=f"psq{j}_{p4}")
                        ps = box[f'q{p4}']
                        nc.tensor.matmul(ps, wq_sb[ct][:, p4 * 128:(p4 + 1) * 128],
                                         xcol(ct, j),
                                         start=(ct == 0), stop=(ct == NCT - 1))
                        if ct == NCT - 1:
                            nc.vector.tensor_copy(qt_sb[p4][:, cols], ps)
                    return f

                def trans(t4, g):
                    def f():
                        t = j * 4 + t4
                        vt = box['vt']
                        pst = psProj.tile([128, TQB], F32, tag="pj",
                                          name=f"pst{j}_{t4}_{g}")
                        nc.tensor.transpose(
                            pst[:, 0:64],
                            vt[g * 64:(g + 1) * 64, t4 * 128:(t4 + 1) * 128],
                            ident[g * 64:(g + 1) * 64, 0:64])
                        nc.vector.tensor_copy(
                            v_sb[g][:, t * 128 + 64:t * 128 + 128],
                            pst[:, 0:64])
                    return f

                for ct in range(NCT):
                    slots.append(k_mm(ct))
                for ct in range(NCT):
                    slots.append(v_mm(ct))
                for ct in range(NCT):
                    slots.append(q_mm(0, ct))
                for ct in range(NCT):
                    slots.append(q_mm(1, ct))
                # interleave transposes with q2/q3 so consecutive psProj
                # allocations are separated by chain matmuls
                ti = [(t4, g) for t4 in range(4) for g in range(NG_LOC)]
                q23 = [(2, ct) for ct in range(NCT)] + [(3, ct) for ct in range(NCT)]
                qi = 0
                for t4, g in ti:
                    if qi < len(q23):
                        slots.append(q_mm(*q23[qi]))
                        qi += 1
                    slots.append(trans(t4, g))
                while qi < len(q23):
                    slots.append(q_mm(*q23[qi]))
                    qi += 1
                return slots

            def yproj_half_slots(j):
                # one slot = one half out-proj chain (4 matmuls + copy [+DMA])
                slots = []
                for tt in range(4):
                    box = {}

                    def mk(half, tt=tt, box=box):
                        def f(pool=None):
                            if half == 0:
                                box['ysb'] = ypool.tile([128, C], F32, tag="y",
                                                        name=f"ysb{j}_{tt}")
                            pl = pool if pool is not None else psY
                            tg = "pj" if pl is psProj else "yp"
                            yp = pl.tile([128, TQB], F32, tag=tg,
                                         name=f"yp{j}_{tt}_{half}")
                            for p4 in range(4):
                                nc.tensor.matmul(
                                    yp,
                                    at_tiles[j][p4][:, tt * 128:(tt + 1) * 128],
                                    wp_sb[p4][:, half * TQB:(half + 1) * TQB],
                                    start=(p4 == 0), stop=(p4 == 3))
                            nc.vector.tensor_copy(
                                box['ysb'][:, half * TQB:(half + 1) * TQB], yp)
                            if half == 1:
                                tau = j * 4 + tt
                                nc.sync.dma_start(
                                    out=y[tau * 128:(tau + 1) * 128, :],
                                    in_=box['ysb'])
                        return f

                    slots.append(mk(0))
                    slots.append(mk(1))
                return slots

            # ---------------- attention ----------------
            PAIRS = [(0, 5), (2, 7), (4, 1), (6, 3)]

            def emit_head_pair(j, hA, hB, fill):
                tq0 = j * TQB
                ntau = 2 * (j + 1)
                cx = {}
                for h in (hA, hB):
                    g, p4, r = h // 4, h // 2, h % 2
                    po = psO.tile([128, TQB], F32, tag="po", name=f"po{h}")
                    cx[h] = (g, p4, r, kdup[g][r * 64:(r + 1) * 64, :],
                             qt_sb[p4][r * 64:(r + 1) * 64, :], po)

                def emit_pvs(entries, stop):
                    for h, t0, t1, pt, off0, off1 in entries:
                        g, p4, r, kT, qT, po = cx[h]
                        nc.tensor.matmul(
                            po[:, off0:TQB],
                            v_sb[g][:, t0 * 128:(t0 + 1) * 128],
                            pt[:, off0:TQB],
                            start=(t0 == 0), stop=False)
                        nc.tensor.matmul(
                            po[:, off1:TQB],
                            v_sb[g][:, t1 * 128:(t1 + 1) * 128],
                            pt[:, TQB + off1:2 * TQB],
                            start=False, stop=stop)

                pend = None
                for tau in range(ntau):
                    t0, t1 = 2 * tau, 2 * tau + 1
                    c0, c1 = t0 - 4 * j, t1 - 4 * j
                    off0, off1 = max(0, c0 * 128), max(0, c1 * 128)
                    new = []
                    for h in (hA, hB):
                        g, p4, r, kT, qT, po = cx[h]
                        sd = psS.tile([128, 2 * TQB], F32, tag="sd",
                                      name=f"sd{h}_{tau}")
                        nc.tensor.matmul(
                            sd[:, off0:TQB],
                            kT[:, t0 * 128:(t0 + 1) * 128],
                            qT[:, tq0 + off0:tq0 + TQB],
                            start=True, stop=True)
                        # the right block is computed full-width on diagonal
                        # pairs so the [128,1024] exp range is contiguous; the
                        # sub-diagonal part is masked or never read by the PVs
                        w1 = off1 if c1 < 0 else 0
                        nc.tensor.matmul(
                            sd[:, TQB + w1:2 * TQB],
                            kT[:, t1 * 128:(t1 + 1) * 128],
                            qT[:, tq0 + w1:tq0 + TQB],
                            start=True, stop=True)
                        if c0 >= 0:
                            nc.vector.tensor_tensor(
                                sd[:, off0:off0 + 128], sd[:, off0:off0 + 128],
                                mbias, op=mybir.AluOpType.add)
                        if c1 >= 0:
                            nc.vector.tensor_tensor(
                                sd[:, TQB + off1:TQB + off1 + 128],
                                sd[:, TQB + off1:TQB + off1 + 128],
                                mbias, op=mybir.AluOpType.add)
                        pt = ppool.tile([128, 2 * TQB], BF16, tag="pt",
                                        name=f"pt{h}_{tau}")
                        nc.scalar.activation(pt[:, off0:2 * TQB],
                                             sd[:, off0:2 * TQB],
                                             EXP, scale=SCALE)
                        new.append((h, t0, t1, pt, off0, off1))
                    if pend is not None:
                        emit_pvs(pend, stop=False)
                    pend = new
                    fill()
                emit_pvs(pend, stop=True)
                for h in (hA, hB):
                    g, p4, r, kT, qT, po = cx[h]
                    rcp = small.tile([128, TQB], F32, tag="recip", name=f"rcp{h}")
                    nc.vector.reciprocal_approx_fast(rcp[0:64, :], po[0:64, :])
                    nc.vector.tensor_mul(
                        at_tiles[j][p4][r * 64:(r + 1) * 64, :],
                        po[64:128, :], rcp[0:64, :])

            # ---------------- schedule ----------------
            for f in proj_slots(0):
                f()

            fq_y = []
            for j in range(NTQB):
                at_tiles[j] = [apool.tile([128, TQB], F32R, tag=f"at{p4}",
                                          name=f"at{j}_{p4}")
                               for p4 in range(4)]
                fq_proj = proj_slots(j + 1) if j + 1 < NTQB else []
                groups = 4 * 2 * (j + 1)          # pairs x ntau
                y_rate = 0.75 if j == NTQB - 1 else 0.0
                st = {'acc': 0.0, 'yacc': 0.0, 'gleft': groups}

                def fill(st=st, fq_proj=fq_proj, y_rate=y_rate):
                    share = len(fq_proj) / st['gleft'] if st['gleft'] else 0.0
                    st['gleft'] -= 1
                    st['acc'] += share
                    n = int(st['acc'])
                    st['acc'] -= n
                    for _ in range(n):
                        if fq_proj:
                            fq_proj.pop(0)()
                    st['yacc'] += y_rate
                    if st['yacc'] >= 1.0 and fq_y:
                        st['yacc'] -= 1.0
                        fq_y.pop(0)()

                for hA, hB in PAIRS:
                    emit_head_pair(j, hA, hB, fill)
                for f in fq_proj:
                    f()
                fq_y.extend(yproj_half_slots(j))
            # tail: alternate psY / psProj banks to avoid WAR stalls
            for i, f in enumerate(fq_y):
                f(pool=(psY if i % 2 == 0 else psProj))

    nc.compile()
    return nc


_NC_CACHE = None


def _get_nc():
    global _NC_CACHE
    if _NC_CACHE is None:
        _NC_CACHE = _build_program()
    return _NC_CACHE


def _make_in_maps(x, Wq, Wk, Wv, Wp):
    in_maps = []
    for core in range(8):
        b, tp = core // 2, core % 2
        hs = slice(tp * NH_LOC, (tp + 1) * NH_LOC)
        gs = slice(tp * NG_LOC, (tp + 1) * NG_LOC)
        in_maps.append({
            "xT": np.ascontiguousarray(x[b].T),
            "wqT": np.ascontiguousarray(
                Wq[hs].transpose(2, 0, 1).reshape(C, S)),
            "wkT": np.ascontiguousarray(
                Wk[gs].transpose(2, 0, 1).reshape(C, NG_LOC * HD)),
            "wvT": np.ascontiguousarray(
                Wv[gs].transpose(2, 0, 1).reshape(C, NG_LOC * HD)),
            "wpT": np.ascontiguousarray(Wp[:, tp * S:(tp + 1) * S].T),
        })
    return in_maps


def kernel(x, Wq, Wk, Wv, Wp, bp, _trace=False):
    x = np.asarray(x, dtype=np.float32)
    nc = _get_nc()
    in_maps = _make_in_maps(
        x, np.asarray(Wq, np.float32), np.asarray(Wk, np.float32),
        np.asarray(Wv, np.float32), np.asarray(Wp, np.float32))
    res = run_bass_kernel_spmd(nc, in_maps, list(range(8)), trace=_trace)
    out = np.empty((B, T, C), dtype=np.float32)
    bp32 = np.asarray(bp, np.float32)
    for b in range(B):
        out[b] = res.results[2 * b]["y"] + res.results[2 * b + 1]["y"] + bp32
    if _trace:
        return out, res
    return out


# revision 26
# speedup vs baseline: 1.0221x; 1.0221x over previous
"""GroupQueryAttention TRN2 Bass kernel (v4: double-block exp pipeline).

Problem: B=4, T=2048, C=1024, H=16 heads, G=4 groups, head_dim=64, causal.
Sharding: 8 cores = 4 batches (DP) x 2 tensor-parallel halves (8 heads /
2 groups each). Host pre-transposes x and weight slices; each core computes
a partial output projection over its 512 attention channels; host sums the
two TP partials per batch and adds the bias.

Hardware model (microbenchmarked on this part):
  - back-to-back matmuls sustain ~2.3 rows/ns (2.4GHz) regardless of
    dtype and of concurrent ACT/DVE/DMA load; but any PE stall drops the
    clock and it takes ~3us of continuous work to ramp back;
  - a sustained [128,512] psum->sbuf exp costs ~690ns (not 530): the ACT
    instruction overhead is ~200ns, so exp work across the causal area
    (~320 blocks) is ~220us -- ABOVE the PE's 186us -- unless batched.

Design:
  - score matmuls write TWO tk blocks into one [128,1024] psum tile
    (2 banks; each matmul stays within a bank) and ONE exp covers both
    blocks -> ACT drops to ~160us. The clipped diagonal leaves a garbage
    gap inside the exp range; its exp output is never read by the PVs.
  - TWO heads processed block-interleaved; PVs consume the exp of the
    PREVIOUS double-block, so the PE never waits on ACT.
  - psum: psS 2x[128,1024] (4 banks) + psO 2 + psProj 1 + psY 1.
  - projections(j+1) are chopped into single-matmul filler slots pulled
    between double-groups; ALL out-projection is deferred and injected
    during attention(3) (which otherwise is ACT-bound), the rest after.
  - ACT runs ONLY exps. DVE does all psum->sbuf copies, diag masks,
    normalize, y copies.
  - dtypes: projections fp32r, attention bf16 (rel err ~1.6e-3).
  - PV lhsT v tiles are [ones64 | v64] so PV emits the softmax
    denominator rows at zero extra PE cost.
"""

import sys
import numpy as np
import ml_dtypes

for _p in ("/opt/trn_rl_repo", "/opt/trn_rl_repo/concourse"):
    if _p not in sys.path:
        sys.path.insert(0, _p)

import concourse.bass as bass  # noqa: E402
import concourse.mybir as mybir  # noqa: E402
from concourse import bacc  # noqa: E402
from concourse.tile import TileContext  # noqa: E402
from concourse.bass_utils import run_bass_kernel_spmd  # noqa: E402
from concourse.masks import make_identity, make_upper_triangular  # noqa: E402

F32 = mybir.dt.float32
F32R = mybir.dt.float32r
BF16 = mybir.dt.bfloat16
BF = ml_dtypes.bfloat16

B, T, C = 4, 2048, 1024
NH, NG, HD = 16, 4, 64
NH_LOC, NG_LOC = 8, 2
S = NH_LOC * HD
TQB = 512
NTQB = 4
NKT = 16
NCT = 8
SCALE = float(HD) ** -0.5
EXP = mybir.ActivationFunctionType.Exp


def _build_program():
    nc = bacc.Bacc("TRN2", target_bir_lowering=False, debug=False, num_devices=8)

    xT = nc.dram_tensor("xT", [C, T], F32R, kind="ExternalInput")
    wqT = nc.dram_tensor("wqT", [C, S], F32R, kind="ExternalInput")
    wkT = nc.dram_tensor("wkT", [C, NG_LOC * HD], F32R, kind="ExternalInput")
    wvT = nc.dram_tensor("wvT", [C, NG_LOC * HD], F32R, kind="ExternalInput")
    wpT = nc.dram_tensor("wpT", [S, C], F32R, kind="ExternalInput")
    y = nc.dram_tensor("y", [T, C], F32, kind="ExternalOutput")

    with TileContext(nc) as tc:
        with tc.tile_pool(name="const", bufs=1) as const_pool, \
             tc.tile_pool(name="persist", bufs=1) as persist, \
             tc.tile_pool(name="vtp", bufs=2) as vtp, \
             tc.tile_pool(name="pp", bufs=5) as ppool, \
             tc.tile_pool(name="attn", bufs=2) as apool, \
             tc.tile_pool(name="sm", bufs=4) as small, \
             tc.tile_pool(name="yo", bufs=4) as ypool, \
             tc.tile_pool(name="psProj", bufs=1, space="PSUM") as psProj, \
             tc.tile_pool(name="psS", bufs=2, space="PSUM") as psS, \
             tc.tile_pool(name="psO", bufs=2, space="PSUM") as psO, \
             tc.tile_pool(name="psY", bufs=1, space="PSUM") as psY:

            # ---- constants ----
            ident = const_pool.tile([128, 64], F32)
            make_identity(nc, ident[0:64, 0:64])
            make_identity(nc, ident[64:128, 0:64], nomemset=False)
            # additive causal bias for diagonal blocks: 0 where tq>=tk,
            # -240 otherwise (exp(-240*0.125)=e^-30 ~ 0); applied to the
            # psum scores BEFORE exp so the PVs depend only on the exp
            mask32 = const_pool.tile([128, 128], F32)
            make_upper_triangular(nc, mask32, val=1.0, diag=True)
            mbias = const_pool.tile([128, 128], F32)
            nc.vector.tensor_scalar_mul(mbias, mask32, 240.0)
            nc.vector.tensor_scalar_add(mbias, mbias, -240.0)

            # ---- persistent SBUF ----
            xAt = [persist.tile([128, (NCT // 2) * TQB], F32R, tag=f"xa{hf}",
                                name=f"xa{hf}") for hf in range(2)]
            xBt = [persist.tile([128, (NCT // 2) * (T - TQB)], F32R, tag=f"xb{hf}",
                                name=f"xb{hf}") for hf in range(2)]
            wq_t = persist.tile([128, NCT * S], F32R, tag="wq", name="wq_t")
            wk_t = persist.tile([128, NCT * NG_LOC * HD], F32R, tag="wk", name="wk_t")
            wv_t = persist.tile([128, NCT * NG_LOC * HD], F32R, tag="wv", name="wv_t")
            wp_t = persist.tile([128, 4 * C], F32R, tag="wp", name="wp_t")
            wq_sb = [wq_t[:, ct * S:(ct + 1) * S] for ct in range(NCT)]
            wk_sb = [wk_t[:, ct * 128:(ct + 1) * 128] for ct in range(NCT)]
            wv_sb = [wv_t[:, ct * 128:(ct + 1) * 128] for ct in range(NCT)]
            wp_sb = [wp_t[:, i * C:(i + 1) * C] for i in range(4)]
            qt_sb = [persist.tile([128, T], BF16, tag=f"qt{i}", name=f"qt{i}")
                     for i in range(4)]
            kdup = [persist.tile([128, T], BF16, tag=f"kd{g}", name=f"kd{g}")
                    for g in range(NG_LOC)]
            v_sb = [persist.tile([128, NKT * 128], BF16, tag=f"v{g}", name=f"v{g}")
                    for g in range(NG_LOC)]
            for g in range(NG_LOC):
                nc.vector.memset(v_sb[g], 1.0)

            # ---- input DMAs (consolidated: Sync issues ~600ns each) ----
            # dram [nct*128, W] -> sbuf [128, (nct W)] with c = ct*128 + p
            def _ld(dst, drsrc, nct):
                nc.sync.dma_start(
                    out=dst[:, :].rearrange("p (ct w) -> p ct w", ct=nct),
                    in_=drsrc.rearrange("(ct p) w -> p ct w", ct=nct))

            _ld(wk_t, wkT[:, :], NCT)
            _ld(wv_t, wvT[:, :], NCT)
            _ld(xAt[0], xT[0:512, 0:TQB], 4)
            _ld(xAt[1], xT[512:1024, 0:TQB], 4)
            _ld(wq_t, wqT[:, :], NCT)
            _ld(xBt[0], xT[0:512, TQB:T], 4)
            _ld(xBt[1], xT[512:1024, TQB:T], 4)
            _ld(wp_t, wpT[:, :], 4)

            def xcol(ct, j):
                if j == 0:
                    return xAt[ct // 4][:, (ct % 4) * TQB:(ct % 4 + 1) * TQB]
                w = T - TQB
                base = (ct % 4) * w + (j - 1) * TQB
                return xBt[ct // 4][:, base:base + TQB]

            at_tiles = {}

            # ---------------- filler slot builders ----------------
            def proj_slots(j, rot=None):
                # rot: rotation of (pool, tag, alloc_width) for psum tiles;
                # j=0 runs inline, so rotate across idle pools to avoid
                # WAR stalls at chain boundaries
                if rot is None:
                    rot = [(psProj, "pj", TQB)]
                rstate = {'i': 0}

                def alloc_ps(name):
                    pool, tag, w = rot[rstate['i'] % len(rot)]
                    rstate['i'] += 1
                    tl = pool.tile([128, w], F32, tag=tag, name=name)
                    return tl[:, 0:TQB] if w != TQB else tl

                slots = []
                cols = slice(j * TQB, (j + 1) * TQB)
                box = {}

                def k_mm(ct):
                    def f():
                        if ct == 0:
                            box['k'] = alloc_ps(f"psk{j}")
                        psk = box['k']
                        nc.tensor.matmul(psk, wk_sb[ct], xcol(ct, j),
                                         start=(ct == 0), stop=(ct == NCT - 1))
                        if ct == NCT - 1:
                            nc.vector.tensor_copy(kdup[0][0:64, cols], psk[0:64, :])
                            nc.vector.tensor_copy(kdup[1][64:128, cols],
                                                  psk[64:128, :])
                            nc.sync.dma_start(out=kdup[0][64:128, cols],
                                              in_=kdup[0][0:64, cols])
                            nc.sync.dma_start(out=kdup[1][0:64, cols],
                                              in_=kdup[1][64:128, cols])
                    return f

                def v_mm(ct):
                    def f():
                        if ct == 0:
                            box['v'] = alloc_ps(f"psv{j}")
                        psv = box['v']
                        nc.tensor.matmul(psv, wv_sb[ct], xcol(ct, j),
                                         start=(ct == 0), stop=(ct == NCT - 1))
                        if ct == NCT - 1:
                            box['vt'] = vtp.tile([128, TQB], F32, tag="vt",
                                                 name=f"vt{j}")
                            nc.vector.tensor_copy(box['vt'], psv)
                    return f

                def q_mm(p4, ct):
                    def f():
                        if ct == 0:
                            box[f'q{p4}'] = alloc_ps(f"psq{j}_{p4}")
                        ps = box[f'q{p4}']
                        nc.tensor.matmul(ps, wq_sb[ct][:, p4 * 128:(p4 + 1) * 128],
                                         xcol(ct, j),
                                         start=(ct == 0), stop=(ct == NCT - 1))
                        if ct == NCT - 1:
                            nc.vector.tensor_copy(qt_sb[p4][:, cols], ps)
                    return f

                def trans(t4, g):
                    def f():
                        t = j * 4 + t4
                        vt = box['vt']
                        pst = alloc_ps(f"pst{j}_{t4}_{g}")
                        nc.tensor.transpose(
                            pst[:, 0:64],
                            vt[g * 64:(g + 1) * 64, t4 * 128:(t4 + 1) * 128],
                            ident[g * 64:(g + 1) * 64, 0:64])
                        nc.vector.tensor_copy(
                            v_sb[g][:, t * 128 + 64:t * 128 + 128],
                            pst[:, 0:64])
                    return f

                for ct in range(NCT):
                    slots.append(k_mm(ct))
                for ct in range(NCT):
                    slots.append(v_mm(ct))
                for ct in range(NCT):
                    slots.append(q_mm(0, ct))
                for ct in range(NCT):
                    slots.append(q_mm(1, ct))
                # interleave transposes with q2/q3 so consecutive psProj
                # allocations are separated by chain matmuls
                ti = [(t4, g) for t4 in range(4) for g in range(NG_LOC)]
                q23 = [(2, ct) for ct in range(NCT)] + [(3, ct) for ct in range(NCT)]
                qi = 0
                for t4, g in ti:
                    if qi < len(q23):
                        slots.append(q_mm(*q23[qi]))
                        qi += 1
                    slots.append(trans(t4, g))
                while qi < len(q23):
                    slots.append(q_mm(*q23[qi]))
                    qi += 1
                return slots

            def yproj_half_slots(j):
                # one slot = one half out-proj chain (4 matmuls + copy [+DMA])
                slots = []
                for tt in range(4):
                    box = {}

                    def mk(half, tt=tt, box=box):
                        def f(pool=None):
                            if half == 0:
                                box['ysb'] = ypool.tile([128, C], F32, tag="y",
                                                        name=f"ysb{j}_{tt}")
                            pl = pool if pool is not None else psY
                            tg = "pj" if pl is psProj else "yp"
                            yp = pl.tile([128, TQB], F32, tag=tg,
                                         name=f"yp{j}_{tt}_{half}")
                            for p4 in range(4):
                                nc.tensor.matmul(
                                    yp,
                                    at_tiles[j][p4][:, tt * 128:(tt + 1) * 128],
                                    wp_sb[p4][:, half * TQB:(half + 1) * TQB],
                                    start=(p4 == 0), stop=(p4 == 3))
                            nc.vector.tensor_copy(
                                box['ysb'][:, half * TQB:(half + 1) * TQB], yp)
                            if half == 1:
                                tau = j * 4 + tt
                                nc.sync.dma_start(
                                    out=y[tau * 128:(tau + 1) * 128, :],
                                    in_=box['ysb'])
                        return f

                    slots.append(mk(0))
                    slots.append(mk(1))
                return slots

            # ---------------- attention ----------------
            PAIRS = [(0, 5), (2, 7), (4, 1), (6, 3)]

            def emit_head_pair(j, hA, hB, fill):
                tq0 = j * TQB
                ntau = 2 * (j + 1)
                cx = {}
                for h in (hA, hB):
                    g, p4, r = h // 4, h // 2, h % 2
                    po = psO.tile([128, TQB], F32, tag="po", name=f"po{h}")
                    cx[h] = (g, p4, r, kdup[g][r * 64:(r + 1) * 64, :],
                             qt_sb[p4][r * 64:(r + 1) * 64, :], po)

                def emit_pvs(entries, stop):
                    for h, t0, t1, pt, off0, off1 in entries:
                        g, p4, r, kT, qT, po = cx[h]
                        nc.tensor.matmul(
                            po[:, off0:TQB],
                            v_sb[g][:, t0 * 128:(t0 + 1) * 128],
                            pt[:, off0:TQB],
                            start=(t0 == 0), stop=False)
                        nc.tensor.matmul(
                            po[:, off1:TQB],
                            v_sb[g][:, t1 * 128:(t1 + 1) * 128],
                            pt[:, TQB + off1:2 * TQB],
                            start=False, stop=stop)

                pend = None
                for tau in range(ntau):
                    t0, t1 = 2 * tau, 2 * tau + 1
                    c0, c1 = t0 - 4 * j, t1 - 4 * j
                    off0, off1 = max(0, c0 * 128), max(0, c1 * 128)
                    new = []
                    for h in (hA, hB):
                        g, p4, r, kT, qT, po = cx[h]
                        sd = psS.tile([128, 2 * TQB], F32, tag="sd",
                                      name=f"sd{h}_{tau}")
                        nc.tensor.matmul(
                            sd[:, off0:TQB],
                            kT[:, t0 * 128:(t0 + 1) * 128],
                            qT[:, tq0 + off0:tq0 + TQB],
                            start=True, stop=True)
                        # the right block is computed full-width on diagonal
                        # pairs so the [128,1024] exp range is contiguous; the
                        # sub-diagonal part is masked or never read by the PVs
                        w1 = off1 if c1 < 0 else 0
                        nc.tensor.matmul(
                            sd[:, TQB + w1:2 * TQB],
                            kT[:, t1 * 128:(t1 + 1) * 128],
                            qT[:, tq0 + w1:tq0 + TQB],
                            start=True, stop=True)
                        if c0 >= 0:
                            nc.vector.tensor_tensor(
                                sd[:, off0:off0 + 128], sd[:, off0:off0 + 128],
                                mbias, op=mybir.AluOpType.add)
                        if c1 >= 0:
                            nc.vector.tensor_tensor(
                                sd[:, TQB + off1:TQB + off1 + 128],
                                sd[:, TQB + off1:TQB + off1 + 128],
                                mbias, op=mybir.AluOpType.add)
                        pt = ppool.tile([128, 2 * TQB], BF16, tag="pt",
                                        name=f"pt{h}_{tau}")
                        nc.scalar.activation(pt[:, off0:2 * TQB],
                                             sd[:, off0:2 * TQB],
                                             EXP, scale=SCALE)
                        new.append((h, t0, t1, pt, off0, off1))
                    if pend is not None:
                        emit_pvs(pend, stop=False)
                    pend = new
                    fill()
                emit_pvs(pend, stop=True)
                for h in (hA, hB):
                    g, p4, r, kT, qT, po = cx[h]
                    rcp = small.tile([128, TQB], F32, tag="recip", name=f"rcp{h}")
                    nc.vector.reciprocal_approx_fast(rcp[0:64, :], po[0:64, :])
                    nc.vector.tensor_mul(
                        at_tiles[j][p4][r * 64:(r + 1) * 64, :],
                        po[64:128, :], rcp[0:64, :])

            # ---------------- schedule ----------------
            for f in proj_slots(0, rot=[(psS, "sd", 2 * TQB), (psO, "po", TQB),
                                        (psY, "yp", TQB), (psProj, "pj", TQB)]):
                f()

            fq_y = []
            for j in range(NTQB):
                at_tiles[j] = [apool.tile([128, TQB], F32R, tag=f"at{p4}",
                                          name=f"at{j}_{p4}")
                               for p4 in range(4)]
                fq_proj = proj_slots(j + 1) if j + 1 < NTQB else []
                groups = 4 * 2 * (j + 1)          # pairs x ntau
                y_rate = 0.75 if j == NTQB - 1 else 0.0
                st = {'acc': 0.0, 'yacc': 0.0, 'gleft': groups}

                def fill(st=st, fq_proj=fq_proj, y_rate=y_rate):
                    share = len(fq_proj) / st['gleft'] if st['gleft'] else 0.0
                    st['gleft'] -= 1
                    st['acc'] += share
                    n = int(st['acc'])
                    st['acc'] -= n
                    for _ in range(n):
                        if fq_proj:
                            fq_proj.pop(0)()
                    st['yacc'] += y_rate
                    if st['yacc'] >= 1.0 and fq_y:
                        st['yacc'] -= 1.0
                        fq_y.pop(0)()

                for hA, hB in PAIRS:
                    emit_head_pair(j, hA, hB, fill)
                for f in fq_proj:
                    f()
                fq_y.extend(yproj_half_slots(j))
            # tail: alternate psY / psProj banks to avoid WAR stalls
            for i, f in enumerate(fq_y):
                f(pool=(psY if i % 2 == 0 else psProj))

    nc.compile()
    return nc


_NC_CACHE = None


def _get_nc():
    global _NC_CACHE
    if _NC_CACHE is None:
        _NC_CACHE = _build_program()
    return _NC_CACHE


def _make_in_maps(x, Wq, Wk, Wv, Wp):
    in_maps = []
    for core in range(8):
        b, tp = core // 2, core % 2
        hs = slice(tp * NH_LOC, (tp + 1) * NH_LOC)
        gs = slice(tp * NG_LOC, (tp + 1) * NG_LOC)
        in_maps.append({
            "xT": np.ascontiguousarray(x[b].T),
            "wqT": np.ascontiguousarray(
                Wq[hs].transpose(2, 0, 1).reshape(C, S)),
            "wkT": np.ascontiguousarray(
                Wk[gs].transpose(2, 0, 1).reshape(C, NG_LOC * HD)),
            "wvT": np.ascontiguousarray(
                Wv[gs].transpose(2, 0, 1).reshape(C, NG_LOC * HD)),
            "wpT": np.ascontiguousarray(Wp[:, tp * S:(tp + 1) * S].T),
        })
    return in_maps


def kernel(x, Wq, Wk, Wv, Wp, bp, _trace=False):
    x = np.asarray(x, dtype=np.float32)
    nc = _get_nc()
    in_maps = _make_in_maps(
        x, np.asarray(Wq, np.float32), np.asarray(Wk, np.float32),
        np.asarray(Wv, np.float32), np.asarray(Wp, np.float32))
    res = run_bass_kernel_spmd(nc, in_maps, list(range(8)), trace=_trace)
    out = np.empty((B, T, C), dtype=np.float32)
    bp32 = np.asarray(bp, np.float32)
    for b in range(B):
        out[b] = res.results[2 * b]["y"] + res.results[2 * b + 1]["y"] + bp32
    if _trace:
        return out, res
    return out
